# revision 1
# baseline (speedup 1.0000x reference)
"""LinearSelfAttention (MobileViT-style) Trainium2 Bass kernel.

Full inputs -> full outputs; internally data-parallel over batch across 8
NeuronCores (2 batch elements per core).

Math restructuring vs the reference:
  qkv rows split: w_q (1 row), W_k (384), W_v (384).
  scores  = softmax_n(w_q @ x[b,:,p,:])            (b_q cancels in softmax)
  context = W_k @ (x @ scores) + b_k               (key tensor never formed)
  out     = (W_out * context[c_in]) @ relu(W_v @ x + b_v) + b_out
so the only large matmuls are the v-projection and the out-projection.
"""

import sys

sys.path.insert(0, "/opt/trn_rl_repo")

import numpy as np
import ml_dtypes

import concourse.bacc as bacc
import concourse.tile as tile
from concourse import mybir
from concourse.bass_utils import run_bass_kernel_spmd

N_CORES = 8
B, C, P, N = 16, 384, 4, 4096
BPC = B // N_CORES          # batches per core
NCH = C // 128              # 128-partition chunks of the channel dim
FD = 512                    # matmul moving-operand free chunk
QW = 2048                   # psum tile width (4 banks)
NQ = N // QW                # psum-width slices per tile

BF16 = mybir.dt.bfloat16
F32 = mybir.dt.float32
AF = mybir.ActivationFunctionType
ALU = mybir.AluOpType
AX = mybir.AxisListType


def build_nc(n_iters: int = 1):
    nc = bacc.Bacc("TRN2", target_bir_lowering=False, debug=False)

    xd = nc.declare_dram_parameter("x", [BPC, C, P, N], BF16, isOutput=False)
    wqd = nc.declare_dram_parameter("wq", [128, NCH, 128], BF16, isOutput=False)
    wvd = nc.declare_dram_parameter("wv", [128, NCH, C], BF16, isOutput=False)
    wkd = nc.declare_dram_parameter("wk", [128, NCH, C], F32, isOutput=False)
    wod = nc.declare_dram_parameter("wo", [128, NCH, C], F32, isOutput=False)
    bvd = nc.declare_dram_parameter("bv", [128, NCH], F32, isOutput=False)
    bkd = nc.declare_dram_parameter("bk", [128, NCH], F32, isOutput=False)
    bod = nc.declare_dram_parameter("bo", [128, NCH], F32, isOutput=False)
    od = nc.declare_dram_parameter("out", [BPC, C, P, N], F32, isOutput=True)

    with tile.TileContext(nc) as tc:
        with (
            tc.tile_pool(name="wts", bufs=1) as wts,
            tc.tile_pool(name="xp", bufs=2) as xp,
            tc.tile_pool(name="esp", bufs=2) as esp,
            tc.tile_pool(name="rvp", bufs=2) as rvp,
            tc.tile_pool(name="tmpp", bufs=2) as tmpp,
            tc.tile_pool(name="outp", bufs=3) as outp,
            tc.tile_pool(name="wosp", bufs=2) as wosp,
            tc.tile_pool(name="small", bufs=4) as small,
            tc.tile_pool(name="psum", bufs=2, space="PSUM") as pmm,
        ):
            wq_sb = wts.tile([128, NCH, 128], BF16)
            nc.sync.dma_start(out=wq_sb[:], in_=wqd[:])
            wv_sb = wts.tile([128, NCH, C], BF16)
            nc.sync.dma_start(out=wv_sb[:], in_=wvd[:])
            wk_sb = wts.tile([128, NCH, C], F32)
            nc.sync.dma_start(out=wk_sb[:], in_=wkd[:])
            wo_sb = wts.tile([128, NCH, C], F32)
            nc.sync.dma_start(out=wo_sb[:], in_=wod[:])
            bv_sb = wts.tile([128, NCH], F32)
            nc.sync.dma_start(out=bv_sb[:], in_=bvd[:])
            bk_sb = wts.tile([128, NCH], F32)
            nc.sync.dma_start(out=bk_sb[:], in_=bkd[:])
            bo_sb = wts.tile([128, NCH], F32)
            nc.sync.dma_start(out=bo_sb[:], in_=bod[:])

            def phase_qv(b, p):
                """Load x, q-matmuls, exp(+Z), xs accumulate, v-matmuls+relu.

                Returns the state the deferred cv/out phase needs.
                """
                # ---- load x tile [384, 4096] as 3 chunks of [128, 4096] ----
                x_sb = xp.tile([128, NCH, N], BF16, tag="x")
                for j in range(NCH):
                    nc.sync.dma_start(
                        out=x_sb[:, j, :], in_=xd[b, j * 128 : (j + 1) * 128, p, :]
                    )

                # ---- q phase: q replicated across partitions, then exp ----
                es_sb = esp.tile([128, N], BF16, tag="es")
                zpart = small.tile([128, NQ], F32, tag="zpart")
                for qi in range(NQ):
                    q_ps = pmm.tile([128, QW], F32, tag="mm")
                    for j in range(NCH):
                        for f2 in range(QW // FD):
                            lo = f2 * FD
                            nc.tensor.matmul(
                                q_ps[:, lo : lo + FD],
                                wq_sb[:, j, :],
                                x_sb[:, j, qi * QW + lo : qi * QW + lo + FD],
                                start=(j == 0),
                                stop=(j == NCH - 1),
                            )
                    nc.scalar.activation(
                        es_sb[:, qi * QW : (qi + 1) * QW],
                        q_ps[:],
                        AF.Exp,
                        accum_out=zpart[:, qi : qi + 1],
                    )

                z_sb = small.tile([128, 1], F32, tag="z")
                nc.vector.tensor_reduce(z_sb, zpart, axis=AX.X, op=ALU.add)
                rz_sb = small.tile([128, 1], F32, tag="rz")
                nc.vector.reciprocal(rz_sb, z_sb)

                # ---- xs = x @ es (fused mul + free-dim accumulate on DVE) ----
                xs_sb = small.tile([128, NCH], F32, tag="xs")
                for j in range(NCH):
                    tmp_sb = tmpp.tile([128, N], BF16, tag="tmp")
                    nc.vector.scalar_tensor_tensor(
                        out=tmp_sb,
                        in0=x_sb[:, j, :],
                        scalar=1.0,
                        in1=es_sb,
                        op0=ALU.bypass,
                        op1=ALU.mult,
                        accum_out=xs_sb[:, j : j + 1],
                    )

                # ---- v = W_v @ x ; relu(v + b_v) -> rv (bf16) ----
                rv_sb = rvp.tile([128, NCH, N], BF16, tag="rv")
                for i in range(NCH):
                    for qi in range(NQ):
                        v_ps = pmm.tile([128, QW], F32, tag="mm")
                        for j in range(NCH):
                            for f2 in range(QW // FD):
                                lo = f2 * FD
                                nc.tensor.matmul(
                                    v_ps[:, lo : lo + FD],
                                    wv_sb[:, j, i * 128 : (i + 1) * 128],
                                    x_sb[:, j, qi * QW + lo : qi * QW + lo + FD],
                                    start=(j == 0),
                                    stop=(j == NCH - 1),
                                )
                        nc.scalar.activation(
                            rv_sb[:, i, qi * QW : (qi + 1) * QW],
                            v_ps[:],
                            AF.Relu,
                            bias=bv_sb[:, i : i + 1],
                        )

                return b, p, rz_sb, xs_sb, rv_sb

            def phase_cvout(state):
                b, p, rz_sb, xs_sb, rv_sb = state
                # ---- cv = W_k @ xs (fp32), scale by 1/Z, + b_k ----
                cv_ps = pmm.tile([128, NCH], F32, tag="mm")
                for i in range(NCH):
                    for j in range(NCH):
                        nc.tensor.matmul(
                            cv_ps[:, i : i + 1],
                            wk_sb[:, j, i * 128 : (i + 1) * 128],
                            xs_sb[:, j : j + 1],
                            start=(j == 0),
                            stop=(j == NCH - 1),
                        )
                cv_sb = small.tile([128, NCH], F32, tag="cv")
                for i in range(NCH):
                    nc.scalar.activation(
                        cv_sb[:, i : i + 1],
                        cv_ps[:, i : i + 1],
                        AF.Identity,
                        bias=bk_sb[:, i : i + 1],
                        scale=rz_sb,
                    )

                # ---- wos = wo * cv[c_in] (per-partition scale) ----
                wos_sb = wosp.tile([128, NCH, C], BF16, tag="wos")
                for j in range(NCH):
                    nc.vector.tensor_scalar(
                        out=wos_sb[:, j, :],
                        in0=wo_sb[:, j, :],
                        scalar1=cv_sb[:, j : j + 1],
                        scalar2=None,
                        op0=ALU.mult,
                    )

                # ---- out = wos.T @ rv + b_out ; drain + DMA out ----
                for i in range(NCH):
                    for qi in range(NQ):
                        o_ps = pmm.tile([128, QW], F32, tag="mm")
                        for j in range(NCH):
                            for f2 in range(QW // FD):
                                lo = f2 * FD
                                nc.tensor.matmul(
                                    o_ps[:, lo : lo + FD],
                                    wos_sb[:, j, i * 128 : (i + 1) * 128],
                                    rv_sb[:, j, qi * QW + lo : qi * QW + lo + FD],
                                    start=(j == 0),
                                    stop=(j == NCH - 1),
                                )
                        ob = outp.tile([128, QW], F32, tag="ob")
                        if (i * NQ + qi) % 2 == 0:
                            nc.scalar.activation(
                                ob, o_ps, AF.Identity, bias=bo_sb[:, i : i + 1]
                            )
                        else:
                            nc.vector.tensor_scalar(
                                out=ob,
                                in0=o_ps,
                                scalar1=bo_sb[:, i : i + 1],
                                scalar2=None,
                                op0=ALU.add,
                            )
                        nc.gpsimd.dma_start(
                            out=od[
                                b,
                                i * 128 : (i + 1) * 128,
                                p,
                                qi * QW : (qi + 1) * QW,
                            ],
                            in_=ob,
                        )

            def all_tiles():
                for b in range(BPC):
                    for p in range(P):
                        phase_cvout(phase_qv(b, p))

            if n_iters == 1:
                all_tiles()
            else:
                with tc.For_i(0, n_iters, 1):
                    all_tiles()

    nc.compile()
    return nc


def prep_weights(w_qkv, b_qkv, w_out, b_out):
    """Host-side rearrangement into the SBUF layouts the kernel DMAs."""
    w_qkv = np.asarray(w_qkv, dtype=np.float32)
    b_qkv = np.asarray(b_qkv, dtype=np.float32)
    w_out = np.asarray(w_out, dtype=np.float32)
    b_out = np.asarray(b_out, dtype=np.float32)

    w_q = w_qkv[0]            # [C]
    w_k = w_qkv[1 : 1 + C]    # [C, C] rows = out channel
    w_v = w_qkv[1 + C :]      # [C, C]

    # wq[k, j, m] = w_q[j*128 + k], replicated over m
    wq = np.broadcast_to(
        w_q.reshape(NCH, 128).T[:, :, None], (128, NCH, 128)
    ).astype(ml_dtypes.bfloat16)
    # wv[k, j, m] = W_v[m, j*128+k]  (lhsT layout: [c_in, c_out])
    wv = np.ascontiguousarray(
        w_v.T.reshape(NCH, 128, C).transpose(1, 0, 2)
    ).astype(ml_dtypes.bfloat16)
    wk = np.ascontiguousarray(
        w_k.T.reshape(NCH, 128, C).transpose(1, 0, 2)
    ).astype(np.float32)
    wo = np.ascontiguousarray(
        w_out.T.reshape(NCH, 128, C).transpose(1, 0, 2)
    ).astype(np.float32)

    bv = np.ascontiguousarray(b_qkv[1 + C :].reshape(NCH, 128).T)
    bk = np.ascontiguousarray(b_qkv[1 : 1 + C].reshape(NCH, 128).T)
    bo = np.ascontiguousarray(b_out.reshape(NCH, 128).T)
    return dict(wq=wq, wv=wv, wk=wk, wo=wo, bv=bv, bk=bk, bo=bo)


def make_in_maps(x, w_qkv, b_qkv, w_out, b_out):
    wts = prep_weights(w_qkv, b_qkv, w_out, b_out)
    x_bf = np.asarray(x, dtype=np.float32).astype(ml_dtypes.bfloat16)
    in_maps = []
    for c in range(N_CORES):
        m = dict(wts)
        m["x"] = np.ascontiguousarray(x_bf[c * BPC : (c + 1) * BPC])
        in_maps.append(m)
    return in_maps


_NC_CACHE = {}


def get_nc(n_iters: int = 1):
    if n_iters not in _NC_CACHE:
        _NC_CACHE[n_iters] = build_nc(n_iters)
    return _NC_CACHE[n_iters]


def kernel(x, w_qkv, b_qkv, w_out, b_out):
    nc = get_nc(1)
    in_maps = make_in_maps(x, w_qkv, b_qkv, w_out, b_out)
    res = run_bass_kernel_spmd(nc, in_maps, core_ids=list(range(N_CORES)))
    out = np.concatenate([res.results[c]["out"] for c in range(N_CORES)], axis=0)
    return out



# revision 2
# speedup vs baseline: 1.1926x; 1.1926x over previous
"""LinearSelfAttention (MobileViT-style) Trainium2 Bass kernel.

Full inputs -> full outputs; internally data-parallel over batch across 8
NeuronCores (2 batch elements per core).

Math restructuring vs the reference:
  qkv rows split: w_q (1 row), W_k (384), W_v (384).
  scores  = softmax_n(w_q @ x[b,:,p,:])            (b_q cancels in softmax)
  context = W_k @ (x @ scores) + b_k               (key tensor never formed)
  out     = (W_out * context[c_in]) @ relu(W_v @ x + b_v) + b_out
so the only large matmuls are the v-projection and the out-projection.
"""

import sys

sys.path.insert(0, "/opt/trn_rl_repo")

import numpy as np
import ml_dtypes

import concourse.bacc as bacc
import concourse.tile as tile
from concourse import mybir
from concourse.bass_utils import run_bass_kernel_spmd

N_CORES = 8
B, C, P, N = 16, 384, 4, 4096
BPC = B // N_CORES          # batches per core
NCH = C // 128              # 128-partition chunks of the channel dim
FD = 512                    # matmul moving-operand free chunk
QW = 2048                   # psum tile width (4 banks)
NQ = N // QW                # psum-width slices per tile

BF16 = mybir.dt.bfloat16
F32 = mybir.dt.float32
AF = mybir.ActivationFunctionType
ALU = mybir.AluOpType
AX = mybir.AxisListType


def build_nc(n_iters: int = 1):
    nc = bacc.Bacc("TRN2", target_bir_lowering=False, debug=False)

    xd = nc.declare_dram_parameter("x", [BPC, C, P, N], BF16, isOutput=False)
    wqd = nc.declare_dram_parameter("wq", [128, NCH, 128], BF16, isOutput=False)
    wvd = nc.declare_dram_parameter("wv", [128, NCH, C], BF16, isOutput=False)
    wkd = nc.declare_dram_parameter("wk", [128, NCH, C], F32, isOutput=False)
    wod = nc.declare_dram_parameter("wo", [128, NCH, C], F32, isOutput=False)
    bvd = nc.declare_dram_parameter("bv", [128, NCH], F32, isOutput=False)
    bkd = nc.declare_dram_parameter("bk", [128, NCH], F32, isOutput=False)
    bod = nc.declare_dram_parameter("bo", [128, NCH], F32, isOutput=False)
    od = nc.declare_dram_parameter("out", [BPC, C, P, N], BF16, isOutput=True)

    with tile.TileContext(nc) as tc:
        with (
            tc.tile_pool(name="wts", bufs=1) as wts,
            tc.tile_pool(name="xp", bufs=2) as xp,
            tc.tile_pool(name="esp", bufs=2) as esp,
            tc.tile_pool(name="rvp", bufs=2) as rvp,
            tc.tile_pool(name="tmpp", bufs=2) as tmpp,
            tc.tile_pool(name="outp", bufs=3) as outp,
            tc.tile_pool(name="wosp", bufs=2) as wosp,
            tc.tile_pool(name="small", bufs=4) as small,
            tc.tile_pool(name="psum", bufs=2, space="PSUM") as pmm,
        ):
            wq_sb = wts.tile([128, NCH, 128], BF16)
            nc.sync.dma_start(out=wq_sb[:], in_=wqd[:])
            wv_sb = wts.tile([128, NCH, C], BF16)
            nc.sync.dma_start(out=wv_sb[:], in_=wvd[:])
            wk_sb = wts.tile([128, NCH, C], F32)
            nc.sync.dma_start(out=wk_sb[:], in_=wkd[:])
            wo_sb = wts.tile([128, NCH, C], F32)
            nc.sync.dma_start(out=wo_sb[:], in_=wod[:])
            bv_sb = wts.tile([128, NCH], F32)
            nc.sync.dma_start(out=bv_sb[:], in_=bvd[:])
            bk_sb = wts.tile([128, NCH], F32)
            nc.sync.dma_start(out=bk_sb[:], in_=bkd[:])
            bo_sb = wts.tile([128, NCH], F32)
            nc.sync.dma_start(out=bo_sb[:], in_=bod[:])

            def phase_qv(b, p):
                """Load x, q-matmuls, exp(+Z), xs accumulate, v-matmuls+relu.

                Returns the state the deferred cv/out phase needs.
                """
                # ---- load x tile [384, 4096] as 3 chunks of [128, 4096] ----
                x_sb = xp.tile([128, NCH, N], BF16, tag="x")
                for j in range(NCH):
                    nc.sync.dma_start(
                        out=x_sb[:, j, :], in_=xd[b, j * 128 : (j + 1) * 128, p, :]
                    )

                # ---- q phase: q replicated across partitions, then exp ----
                es_sb = esp.tile([128, N], BF16, tag="es")
                zpart = small.tile([128, NQ], F32, tag="zpart")
                for qi in range(NQ):
                    q_ps = pmm.tile([128, QW], F32, tag="mm")
                    for j in range(NCH):
                        for f2 in range(QW // FD):
                            lo = f2 * FD
                            nc.tensor.matmul(
                                q_ps[:, lo : lo + FD],
                                wq_sb[:, j, :],
                                x_sb[:, j, qi * QW + lo : qi * QW + lo + FD],
                                start=(j == 0),
                                stop=(j == NCH - 1),
                            )
                    nc.scalar.activation(
                        es_sb[:, qi * QW : (qi + 1) * QW],
                        q_ps[:],
                        AF.Exp,
                        accum_out=zpart[:, qi : qi + 1],
                    )

                z_sb = small.tile([128, 1], F32, tag="z")
                nc.vector.tensor_reduce(z_sb, zpart, axis=AX.X, op=ALU.add)
                rz_sb = small.tile([128, 1], F32, tag="rz")
                nc.vector.reciprocal(rz_sb, z_sb)

                # ---- xs = x @ es (fused mul + free-dim accumulate on DVE) ----
                xs_sb = small.tile([128, NCH], F32, tag="xs")
                for j in range(NCH):
                    tmp_sb = tmpp.tile([128, N], BF16, tag="tmp")
                    nc.vector.scalar_tensor_tensor(
                        out=tmp_sb,
                        in0=x_sb[:, j, :],
                        scalar=1.0,
                        in1=es_sb,
                        op0=ALU.bypass,
                        op1=ALU.mult,
                        accum_out=xs_sb[:, j : j + 1],
                    )

                # ---- v = W_v @ x ; relu(v + b_v) -> rv (bf16) ----
                rv_sb = rvp.tile([128, NCH, N], BF16, tag="rv")
                for i in range(NCH):
                    for qi in range(NQ):
                        v_ps = pmm.tile([128, QW], F32, tag="mm")
                        for j in range(NCH):
                            for f2 in range(QW // FD):
                                lo = f2 * FD
                                nc.tensor.matmul(
                                    v_ps[:, lo : lo + FD],
                                    wv_sb[:, j, i * 128 : (i + 1) * 128],
                                    x_sb[:, j, qi * QW + lo : qi * QW + lo + FD],
                                    start=(j == 0),
                                    stop=(j == NCH - 1),
                                )
                        nc.scalar.activation(
                            rv_sb[:, i, qi * QW : (qi + 1) * QW],
                            v_ps[:],
                            AF.Relu,
                            bias=bv_sb[:, i : i + 1],
                        )

                return b, p, rz_sb, xs_sb, rv_sb

            def phase_cvout(state):
                b, p, rz_sb, xs_sb, rv_sb = state
                # ---- cv = W_k @ xs (fp32), scale by 1/Z, + b_k ----
                cv_ps = pmm.tile([128, NCH], F32, tag="mm")
                for i in range(NCH):
                    for j in range(NCH):
                        nc.tensor.matmul(
                            cv_ps[:, i : i + 1],
                            wk_sb[:, j, i * 128 : (i + 1) * 128],
                            xs_sb[:, j : j + 1],
                            start=(j == 0),
                            stop=(j == NCH - 1),
                        )
                cv_sb = small.tile([128, NCH], F32, tag="cv")
                for i in range(NCH):
                    nc.scalar.activation(
                        cv_sb[:, i : i + 1],
                        cv_ps[:, i : i + 1],
                        AF.Identity,
                        bias=bk_sb[:, i : i + 1],
                        scale=rz_sb,
                    )

                # ---- wos = wo * cv[c_in] (per-partition scale) ----
                wos_sb = wosp.tile([128, NCH, C], BF16, tag="wos")
                for j in range(NCH):
                    nc.vector.tensor_scalar(
                        out=wos_sb[:, j, :],
                        in0=wo_sb[:, j, :],
                        scalar1=cv_sb[:, j : j + 1],
                        scalar2=None,
                        op0=ALU.mult,
                    )

                # ---- out = wos.T @ rv + b_out ; drain + DMA out ----
                for i in range(NCH):
                    for qi in range(NQ):
                        o_ps = pmm.tile([128, QW], F32, tag="mm")
                        for j in range(NCH):
                            for f2 in range(QW // FD):
                                lo = f2 * FD
                                nc.tensor.matmul(
                                    o_ps[:, lo : lo + FD],
                                    wos_sb[:, j, i * 128 : (i + 1) * 128],
                                    rv_sb[:, j, qi * QW + lo : qi * QW + lo + FD],
                                    start=(j == 0),
                                    stop=(j == NCH - 1),
                                )
                        ob = outp.tile([128, QW], BF16, tag="ob")
                        if (i * NQ + qi) % 2 == 0:
                            nc.scalar.activation(
                                ob, o_ps, AF.Identity, bias=bo_sb[:, i : i + 1]
                            )
                        else:
                            nc.vector.tensor_scalar(
                                out=ob,
                                in0=o_ps,
                                scalar1=bo_sb[:, i : i + 1],
                                scalar2=None,
                                op0=ALU.add,
                            )
                        nc.gpsimd.dma_start(
                            out=od[
                                b,
                                i * 128 : (i + 1) * 128,
                                p,
                                qi * QW : (qi + 1) * QW,
                            ],
                            in_=ob,
                        )

            def all_tiles():
                for b in range(BPC):
                    for p in range(P):
                        phase_cvout(phase_qv(b, p))

            if n_iters == 1:
                all_tiles()
            else:
                with tc.For_i(0, n_iters, 1):
                    all_tiles()

    nc.compile()
    return nc


def prep_weights(w_qkv, b_qkv, w_out, b_out):
    """Host-side rearrangement into the SBUF layouts the kernel DMAs."""
    w_qkv = np.asarray(w_qkv, dtype=np.float32)
    b_qkv = np.asarray(b_qkv, dtype=np.float32)
    w_out = np.asarray(w_out, dtype=np.float32)
    b_out = np.asarray(b_out, dtype=np.float32)

    w_q = w_qkv[0]            # [C]
    w_k = w_qkv[1 : 1 + C]    # [C, C] rows = out channel
    w_v = w_qkv[1 + C :]      # [C, C]

    # wq[k, j, m] = w_q[j*128 + k], replicated over m
    wq = np.broadcast_to(
        w_q.reshape(NCH, 128).T[:, :, None], (128, NCH, 128)
    ).astype(ml_dtypes.bfloat16)
    # wv[k, j, m] = W_v[m, j*128+k]  (lhsT layout: [c_in, c_out])
    wv = np.ascontiguousarray(
        w_v.T.reshape(NCH, 128, C).transpose(1, 0, 2)
    ).astype(ml_dtypes.bfloat16)
    wk = np.ascontiguousarray(
        w_k.T.reshape(NCH, 128, C).transpose(1, 0, 2)
    ).astype(np.float32)
    wo = np.ascontiguousarray(
        w_out.T.reshape(NCH, 128, C).transpose(1, 0, 2)
    ).astype(np.float32)

    bv = np.ascontiguousarray(b_qkv[1 + C :].reshape(NCH, 128).T)
    bk = np.ascontiguousarray(b_qkv[1 : 1 + C].reshape(NCH, 128).T)
    bo = np.ascontiguousarray(b_out.reshape(NCH, 128).T)
    return dict(wq=wq, wv=wv, wk=wk, wo=wo, bv=bv, bk=bk, bo=bo)


def make_in_maps(x, w_qkv, b_qkv, w_out, b_out):
    wts = prep_weights(w_qkv, b_qkv, w_out, b_out)
    x_bf = np.asarray(x, dtype=np.float32).astype(ml_dtypes.bfloat16)
    in_maps = []
    for c in range(N_CORES):
        m = dict(wts)
        m["x"] = np.ascontiguousarray(x_bf[c * BPC : (c + 1) * BPC])
        in_maps.append(m)
    return in_maps


_NC_CACHE = {}


def get_nc(n_iters: int = 1):
    if n_iters not in _NC_CACHE:
        _NC_CACHE[n_iters] = build_nc(n_iters)
    return _NC_CACHE[n_iters]


def kernel(x, w_qkv, b_qkv, w_out, b_out):
    nc = get_nc(1)
    in_maps = make_in_maps(x, w_qkv, b_qkv, w_out, b_out)
    res = run_bass_kernel_spmd(nc, in_maps, core_ids=list(range(N_CORES)))
    out = np.concatenate([res.results[c]["out"] for c in range(N_CORES)], axis=0)
    return out.astype(np.float32)

